# revision 27
# baseline (speedup 1.0000x reference)
"""ConvGNN message-passing kernel for 8x Trainium2 NeuronCores.

Problem (hardcoded):
    batch:         (4, 50000, 64)  f32
    neighborhoods: (50000, 16)     int64, values in [0, 50000] (50000 = zero node)
    kernel:        (16, 64, 64)    f32
    bias:          (1, 1, 64)      f32
    out[b, n, :] = sum_k  x[b, nb[n, k], :] @ W[k]  + bias

Strategy:
  * Host packs the node table to node-major bf16 rows of (B*CIN)=256 units
    (512 B). 8 cores shard the 50000 output nodes (6250 each); neighborhood
    indices are identical across b, so one gathered row serves all 4 batches.
  * dma_gather indices are int16 (< 32768), but the table needs 50001 rows.
    Instead of gathering twice from two table halves (2x traffic), each tile
    t gathers from a sliding window table[S*t : S*t + 32768) and the host
    PERMUTES table rows so that every node referenced by tile t lands inside
    tile t's window: node with tile-span [t_min, t_max] gets a slot in
    [S*t_max, S*t_min + 32768) via greedy interval assignment (~86% load, no
    duplication). One gather per position -> half the traffic of the 2-pass.
  * On-device gather uses nc.gpsimd.dma_gather(transpose=True) from HBM: the
    DMA xbar sprays each 512B row across 128 partitions, landing data in
    [(b01, c) partition, position] layout = the matmul rhs orientation.
  * PE: per tile, 16 accumulating matmuls (one per k) with block-diag
    [[Wk,0],[0,Wk]] stationary tiles over both slabs at once:
    psum[(2b, o) x (slab, node)] in a single 2KB PSUM bank.
  * Tiles: 24 x 256 nodes + 104 + 8 (6 pad): the tiny final tile keeps the
    end-of-kernel gather->PE->evac->write chain short. Gathers are
    five-way buffered; idx preload is chunked so the first gather starts
    immediately; output staged in bf16.
  * Output written as [slab, (2b, o), n] bf16; host unpacks to (B, N, COUT).
"""

import numpy as np
import ml_dtypes

import concourse.bacc as bacc
import concourse.bass as bass
import concourse.mybir as mybir
from concourse.bass_utils import run_bass_kernel_spmd
from concourse.library_config import mlp

# ---------------------------------------------------------------- constants
B, N, K, CIN, COUT = 4, 50000, 16, 64, 64
NCORES = 8
NODES_PER_CORE = N // NCORES          # 6250
NODE_TILE = 256                       # nodes per full tile

TN = [NODE_TILE] * 24 + [104, 8]      # nodes per tile (all %8, pos %128)
NT = len(TN)                          # 26
NOFF = [0]
for _n in TN:
    NOFF.append(NOFF[-1] + _n)
NPAD = NOFF[-1]                       # 6256 padded nodes per core

WIN = 32768                           # int16 gather window (rows)
S = 704                               # window stride per tile
R = (NT - 1) * S + WIN                # 50368 table rows per core
ZERO = N                              # vnode for the zero row (jax index N)
ELEM = B * CIN                        # 256 bf16 units per row (512 B)
SLABS = ELEM // 128                   # 2
POS_TILE = NODE_TILE * K              # 4096 positions per full tile

# idx preload chunks (tile-aligned col ranges; cols per tile == nodes)
IDX_CHUNKS = [(NOFF[0], NOFF[1]), (NOFF[1], NOFF[13]), (NOFF[13], NOFF[NT])]
CHUNK_WAIT = {0: 0, 1: 1, 13: 2}      # tile index -> isem index to wait on

GBUFS = 5                             # gather buffers for the full tiles

F32 = mybir.dt.float32
BF16 = mybir.dt.bfloat16
I16 = mybir.dt.int16

BF = ml_dtypes.bfloat16


# ---------------------------------------------------------------- program
def build_program():
    nc = bacc.Bacc("TRN2")

    table = nc.declare_dram_parameter("table", [R, ELEM], BF16, isOutput=False)
    idx_d = nc.declare_dram_parameter("idx", [128, NPAD], I16, isOutput=False)
    w_d = nc.declare_dram_parameter("w", [128, K * 128], BF16, isOutput=False)
    bias_d = nc.declare_dram_parameter("biasx", [128, 1], F32, isOutput=False)
    out_d = nc.declare_dram_parameter("out", [SLABS, 128, NPAD], BF16, isOutput=True)

    from contextlib import ExitStack
    with ExitStack() as ctx:
        e = ctx.enter_context
        idx_sb = e(nc.sbuf_tensor("idx_sb", [128, NPAD], I16))
        w_sb = e(nc.sbuf_tensor("w_sb", [128, K * 128], BF16))
        bias_sb = e(nc.sbuf_tensor("bias_sb", [128, 1], F32))
        g = [e(nc.sbuf_tensor(f"g{j}", [128, SLABS, POS_TILE], BF16))
             for j in range(GBUFS)]
        gsp = {24: e(nc.sbuf_tensor("gsp24", [128, SLABS, TN[24] * K], BF16)),
               25: e(nc.sbuf_tensor("gsp25", [128, SLABS, TN[25] * K], BF16))}
        stage = [e(nc.sbuf_tensor(f"stage{j}", [128, SLABS, NODE_TILE], BF16))
                 for j in range(2)]
        psum = [e(nc.psum_tensor(f"ps{j}", [128, SLABS, NODE_TILE], F32))
                for j in range(2)]
        pssp = {24: e(nc.psum_tensor("psl24", [128, SLABS, TN[24]], F32)),
                25: e(nc.psum_tensor("psl25", [128, SLABS, TN[25]], F32))}
        isem = [e(nc.semaphore(f"isem{j}")) for j in range(len(IDX_CHUNKS))]
        wsem = e(nc.semaphore("wsem"))
        bsem = e(nc.semaphore("bsem"))
        gsem = [e(nc.semaphore(f"gsem{j}")) for j in range(GBUFS)]
        gspsem = {24: e(nc.semaphore("gsem24")), 25: e(nc.semaphore("gsem25"))}
        mm_sem = e(nc.semaphore("mm_sem"))
        evac_sem = e(nc.semaphore("evac_sem"))
        osem = [[e(nc.semaphore(f"osem{j}{s}")) for s in range(SLABS)]
                for j in range(2)]
        block = e(nc.Block())

        def gbuf(i):
            return gsp[i] if i in gsp else g[i % GBUFS]

        def pbuf(i):
            return pssp[i] if i in pssp else psum[i % 2]

        @block.sync
        def _(sync):
            c0 = IDX_CHUNKS[0]
            sync.dma_start(out=idx_sb[:, c0[0]:c0[1]],
                           in_=idx_d[:, c0[0]:c0[1]]).then_inc(isem[0], 16)
            sync.dma_start(out=w_sb[:, :], in_=w_d[:, :]).then_inc(wsem, 16)
            sync.dma_start(out=bias_sb[:, :], in_=bias_d[:, :]).then_inc(bsem, 16)
            for j, c in enumerate(IDX_CHUNKS[1:], start=1):
                sync.dma_start(out=idx_sb[:, c[0]:c[1]],
                               in_=idx_d[:, c[0]:c[1]]).then_inc(isem[j], 16)
            for i in range(NT):
                buf = i % 2
                nn = TN[i]
                sync.wait_ge(evac_sem, i + 1)
                for s in range(SLABS):
                    sync.dma_start(
                        out=out_d[s, :, NOFF[i]:NOFF[i] + nn],
                        in_=stage[buf][:, s, 0:nn],
                    ).then_inc(osem[buf][s], 16)
            for j in range(2):
                cnt = len([i for i in range(NT) if i % 2 == j])
                for s in range(SLABS):
                    sync.wait_ge(osem[j][s], cnt * 16)

        @block.gpsimd
        def _(gpsimd):
            gpsimd.load_library(mlp)
            for i in range(NT):
                npos = TN[i] * K
                if i in CHUNK_WAIT:
                    gpsimd.wait_ge(isem[CHUNK_WAIT[i]], 16)
                if i >= GBUFS and i not in gsp:
                    gpsimd.wait_ge(mm_sem, i - GBUFS + 1)
                sl = slice(NOFF[i], NOFF[i + 1])
                sem = gspsem[i] if i in gsp else gsem[i % GBUFS]
                gpsimd.dma_gather(
                    out_ap=gbuf(i)[:, :, :],
                    in_ap=table[S * i:S * i + WIN, :],
                    idxs_ap=idx_sb[:, sl],
                    num_idxs=npos,
                    num_idxs_reg=npos,
                    elem_size=ELEM,
                    elem_step=ELEM,
                    transpose=True,
                    single_packet=False,
                ).then_inc(sem, 16)

        @block.vector
        def _(vector):
            vector.wait_ge(bsem, 16)
            for i in range(NT):
                buf = i % 2
                nn = TN[i]
                vector.wait_ge(mm_sem, i + 1)
                if i >= 2:
                    for s in range(SLABS):
                        vector.wait_ge(osem[buf][s], 16 * ((i - 2) // 2 + 1))
                vector.tensor_add(
                    stage[buf][:, :, 0:nn],
                    pbuf(i)[:, :, :],
                    bias_sb[:, :].to_broadcast([128, SLABS, nn]),
                ).then_inc(evac_sem, 1)

        @block.tensor
        def _(tensor):
            tensor.wait_ge(wsem, 16)
            for i in range(NT):
                nn = TN[i]
                if i in gsp:
                    tensor.wait_ge(gspsem[i], 16)
                else:
                    tensor.wait_ge(gsem[i % GBUFS], 16 * (i // GBUFS + 1))
                if i >= 2 and i not in gsp:
                    tensor.wait_ge(evac_sem, i - 1)
                src = gbuf(i)
                ps = pbuf(i)
                for k in range(K):
                    ins = tensor.matmul(
                        ps[:, :, :],
                        w_sb[:, k * 128:(k + 1) * 128],
                        src[:, :, k * nn:(k + 1) * nn],
                        start=(k == 0),
                        stop=(k == K - 1),
                    )
                ins.then_inc(mm_sem, 1)

    nc.compile()
    return nc


# ---------------------------------------------------------------- host side
_TILE_OF_NODE = np.zeros(NPAD, dtype=np.int32)
for _t in range(NT):
    _TILE_OF_NODE[NOFF[_t]:NOFF[_t + 1]] = _t


def _assign_slots(refs):
    """refs: [NPAD, K] int32 vnodes. Returns (slot[N+1], spill dict v->{t:slot}).

    Each referenced vnode needs one table slot usable by every tile that
    references it: slot in [S*t_max, S*t_min + WIN). Greedy by deadline with
    union-find next-free; rare infeasible nodes fall back to per-tile slots.
    """
    tile_of_pos = np.repeat(_TILE_OF_NODE, K)
    vn = refs.reshape(-1)

    tmin = np.full(N + 1, NT, dtype=np.int32)
    tmax = np.full(N + 1, -1, dtype=np.int32)
    np.minimum.at(tmin, vn, tile_of_pos)
    np.maximum.at(tmax, vn, tile_of_pos)
    nodes = np.nonzero(tmax >= 0)[0]
    lo = (S * tmax[nodes]).astype(np.int64)
    hi = (S * tmin[nodes] + WIN).astype(np.int64)
    order = np.argsort(hi, kind="stable")

    parent = np.arange(R + 1, dtype=np.int64)

    def find(x):
        root = x
        while parent[root] != root:
            root = parent[root]
        while parent[x] != root:
            parent[x], x = root, parent[x]
        return root

    slot = np.full(N + 1, -1, dtype=np.int64)
    spilled = []
    for i in order:
        f = find(lo[i])
        if f >= hi[i]:
            spilled.append(nodes[i])
            continue
        slot[nodes[i]] = f
        parent[f] = f + 1

    spill = {}
    for v in spilled:
        # per-tile fallback: give the node a slot inside each referencing
        # tile's own window (always feasible while the table isn't full)
        spill[v] = {}
        for t in np.unique(tile_of_pos[vn == v]):
            f = find(S * t)
            if f >= S * t + WIN:
                raise RuntimeError("gather table window full")
            spill[v][int(t)] = int(f)
            parent[f] = f + 1
    return slot, spill


def _pack_inputs(batch, neighborhoods, kernel, bias):
    batch = np.asarray(batch, dtype=np.float32)
    nb = np.asarray(neighborhoods, dtype=np.int64).astype(np.int32)
    w = np.asarray(kernel, dtype=np.float32)
    bias = np.asarray(bias, dtype=np.float32).reshape(COUT)

    # node content rows: (N, 256) bf16, unit u = batch b=u//64, channel u%64
    noderows = batch.transpose(1, 0, 2).reshape(N, ELEM).astype(BF)

    # block-diag stationary weight tiles [128, K*128]
    wt = np.zeros((128, K, 128), dtype=BF)
    wbf = w.astype(BF)
    for k in range(K):
        wt[0:64, k, 0:64] = wbf[k]
        wt[64:128, k, 64:128] = wbf[k]
    wt = wt.reshape(128, K * 128)

    bias_t = np.tile(bias, 2).reshape(128, 1).astype(np.float32)

    tables = []
    idx_maps = []
    for c in range(NCORES):
        n0 = c * NODES_PER_CORE
        refs = np.full((NPAD, K), ZERO, dtype=np.int32)
        refs[:NODES_PER_CORE] = nb[n0:n0 + NODES_PER_CORE]

        slot, spill = _assign_slots(refs)

        table = np.zeros((R, ELEM), dtype=BF)
        assigned = np.nonzero(slot >= 0)[0]
        real = assigned[assigned < N]
        table[slot[real]] = noderows[real]
        for v, tslots in spill.items():
            if v < N:
                for t, f in tslots.items():
                    table[f] = noderows[v]

        idx = slot[refs] - (S * _TILE_OF_NODE[:, None]).astype(np.int64)
        for v, tslots in spill.items():
            mask = refs == v
            for t, f in tslots.items():
                rows = slice(NOFF[t], NOFF[t + 1])
                idx[rows][mask[rows]] = f - S * t
        assert idx.min() >= 0 and idx.max() < WIN, (idx.min(), idx.max())
        idx = idx.astype(np.int16)

        # wrap positions k-major per tile -> [128, NPAD] int16
        cols = []
        for t in range(NT):
            nn = TN[t]
            a = idx[NOFF[t]:NOFF[t + 1]]                 # [nn, K]
            a = a.T.reshape(nn * K)                      # k-major positions
            a = a.reshape(nn * K // 16, 16).T            # [16, cols]
            cols.append(np.tile(a, (8, 1)))              # [128, cols]
        idx_maps.append(np.ascontiguousarray(np.concatenate(cols, axis=1)))
        tables.append(table)

    return tables, wt, bias_t, idx_maps


_PROGRAM_CACHE = {}


def _run(batch, neighborhoods, kernel, bias, **spmd_kwargs):
    tables, wt, bias_t, idx_maps = _pack_inputs(batch, neighborhoods, kernel, bias)

    if "nc" not in _PROGRAM_CACHE:
        _PROGRAM_CACHE["nc"] = build_program()
    nc = _PROGRAM_CACHE["nc"]

    in_maps = []
    for c in range(NCORES):
        in_maps.append({
            "table": tables[c],
            "idx": idx_maps[c],
            "w": wt,
            "biasx": bias_t,
        })

    kres = run_bass_kernel_spmd(nc, in_maps, list(range(NCORES)), **spmd_kwargs)
    res = kres.results

    out = np.empty((B, N, COUT), dtype=np.float32)
    for c in range(NCORES):
        o = np.asarray(res[c]["out"])[:, :, :NODES_PER_CORE]   # [S, 128, n]
        o = o.astype(np.float32)
        o = o.reshape(SLABS, 2, COUT, NODES_PER_CORE)          # [s, b01, o, n]
        o = o.transpose(0, 1, 3, 2).reshape(B, NODES_PER_CORE, COUT)
        out[:, c * NODES_PER_CORE:(c + 1) * NODES_PER_CORE, :] = o
    return out, kres


def kernel(batch, neighborhoods, kernel, bias):
    out, _ = _run(batch, neighborhoods, kernel, bias)
    return out


# revision 28
# speedup vs baseline: 1.0135x; 1.0135x over previous
"""ConvGNN message-passing kernel for 8x Trainium2 NeuronCores.

Problem (hardcoded):
    batch:         (4, 50000, 64)  f32
    neighborhoods: (50000, 16)     int64, values in [0, 50000] (50000 = zero node)
    kernel:        (16, 64, 64)    f32
    bias:          (1, 1, 64)      f32
    out[b, n, :] = sum_k  x[b, nb[n, k], :] @ W[k]  + bias

Strategy:
  * Host packs the node table to node-major bf16 rows of (B*CIN)=256 units
    (512 B). 8 cores shard the 50000 output nodes (6250 each); neighborhood
    indices are identical across b, so one gathered row serves all 4 batches.
  * dma_gather indices are int16 (< 32768), but the table needs 50001 rows.
    Instead of gathering twice from two table halves (2x traffic), each tile
    t gathers from a sliding window table[S*t : S*t + 32768) and the host
    PERMUTES table rows so that every node referenced by tile t lands inside
    tile t's window: node with tile-span [t_min, t_max] gets a slot in
    [S*t_max, S*t_min + 32768) via greedy interval assignment (~84% load, no
    duplication). One gather per position -> half the traffic of the 2-pass.
  * On-device gather uses nc.gpsimd.dma_gather(transpose=True) from HBM: the
    DMA xbar sprays each 512B row across 128 partitions, landing data in
    [(b01, c) partition, position] layout = the matmul rhs orientation.
  * PE: per tile of 256 nodes, 2 slabs x 16 k accumulating matmuls with
    block-diag [[Wk,0],[0,Wk]] stationary tiles -> psum[(2b, o) x 256 nodes].
    Per-slab semaphores let evac (DVE, +bias, ->bf16) and the output DMA of
    slab 0 overlap the slab-1 matmuls, shortening the end-of-kernel chain.
  * Tiles: 24 x 256 nodes + one 112-node tail (6 pad); gathers five-way
    buffered; idx preload chunked so the first gather starts immediately.
  * Output written as [slab, (2b, o), n] bf16; host unpacks to (B, N, COUT).
"""

import numpy as np
import ml_dtypes

import concourse.bacc as bacc
import concourse.bass as bass
import concourse.mybir as mybir
from concourse.bass_utils import run_bass_kernel_spmd
from concourse.library_config import mlp

# ---------------------------------------------------------------- constants
B, N, K, CIN, COUT = 4, 50000, 16, 64, 64
NCORES = 8
NODES_PER_CORE = N // NCORES          # 6250
NODE_TILE = 256                       # nodes per full tile

TN = [NODE_TILE] * 24 + [112]         # nodes per tile (all %8, pos %128)
NT = len(TN)                          # 25
NOFF = [0]
for _n in TN:
    NOFF.append(NOFF[-1] + _n)
NPAD = NOFF[-1]                       # 6256 padded nodes per core

WIN = 32768                           # int16 gather window (rows)
S = 768                               # window stride per tile
R = (NT - 1) * S + WIN                # 51200 table rows per core
ZERO = N                              # vnode for the zero row (jax index N)
ELEM = B * CIN                        # 256 bf16 units per row (512 B)
SLABS = ELEM // 128                   # 2
POS_TILE = NODE_TILE * K              # 4096 positions per full tile

# idx preload chunks (tile-aligned col ranges; cols per tile == nodes)
IDX_CHUNKS = [(NOFF[0], NOFF[1]), (NOFF[1], NOFF[13]), (NOFF[13], NOFF[NT])]
CHUNK_WAIT = {0: 0, 1: 1, 13: 2}      # tile index -> isem index to wait on

GBUFS = 5                             # gather buffers for the full tiles

F32 = mybir.dt.float32
BF16 = mybir.dt.bfloat16
I16 = mybir.dt.int16

BF = ml_dtypes.bfloat16


# ---------------------------------------------------------------- program
def build_program():
    nc = bacc.Bacc("TRN2")

    table = nc.declare_dram_parameter("table", [R, ELEM], BF16, isOutput=False)
    idx_d = nc.declare_dram_parameter("idx", [128, NPAD], I16, isOutput=False)
    w_d = nc.declare_dram_parameter("w", [128, K * 128], BF16, isOutput=False)
    bias_d = nc.declare_dram_parameter("biasx", [128, 1], F32, isOutput=False)
    out_d = nc.declare_dram_parameter("out", [SLABS, 128, NPAD], BF16, isOutput=True)

    from contextlib import ExitStack
    with ExitStack() as ctx:
        e = ctx.enter_context
        idx_sb = e(nc.sbuf_tensor("idx_sb", [128, NPAD], I16))
        w_sb = e(nc.sbuf_tensor("w_sb", [128, K * 128], BF16))
        bias_sb = e(nc.sbuf_tensor("bias_sb", [128, 1], F32))
        g = [e(nc.sbuf_tensor(f"g{j}", [128, SLABS, POS_TILE], BF16))
             for j in range(GBUFS)]
        gl = e(nc.sbuf_tensor("gl", [128, SLABS, TN[NT - 1] * K], BF16))
        stage = [e(nc.sbuf_tensor(f"stage{j}", [128, SLABS, NODE_TILE], BF16))
                 for j in range(2)]
        psum = [[e(nc.psum_tensor(f"ps{j}{s}", [128, NODE_TILE], F32))
                 for s in range(SLABS)] for j in range(2)]
        psl = [e(nc.psum_tensor(f"psl{s}", [128, TN[NT - 1]], F32))
               for s in range(SLABS)]
        isem = [e(nc.semaphore(f"isem{j}")) for j in range(len(IDX_CHUNKS))]
        wsem = e(nc.semaphore("wsem"))
        bsem = e(nc.semaphore("bsem"))
        gsem = [e(nc.semaphore(f"gsem{j}")) for j in range(GBUFS)]
        glsem = e(nc.semaphore("glsem"))
        mm_sem = e(nc.semaphore("mm_sem"))
        evac_sem = e(nc.semaphore("evac_sem"))
        osem = [[e(nc.semaphore(f"osem{j}{s}")) for s in range(SLABS)]
                for j in range(2)]
        block = e(nc.Block())

        def gbuf(i):
            return gl if i == NT - 1 else g[i % GBUFS]

        def pbuf(i):
            return psl if i == NT - 1 else psum[i % 2]

        @block.sync
        def _(sync):
            c0 = IDX_CHUNKS[0]
            sync.dma_start(out=idx_sb[:, c0[0]:c0[1]],
                           in_=idx_d[:, c0[0]:c0[1]]).then_inc(isem[0], 16)
            sync.dma_start(out=w_sb[:, :], in_=w_d[:, :]).then_inc(wsem, 16)
            sync.dma_start(out=bias_sb[:, :], in_=bias_d[:, :]).then_inc(bsem, 16)
            for j, c in enumerate(IDX_CHUNKS[1:], start=1):
                sync.dma_start(out=idx_sb[:, c[0]:c[1]],
                               in_=idx_d[:, c[0]:c[1]]).then_inc(isem[j], 16)
            for i in range(NT):
                buf = i % 2
                nn = TN[i]
                for s in range(SLABS):
                    sync.wait_ge(evac_sem, SLABS * i + s + 1)
                    sync.dma_start(
                        out=out_d[s, :, NOFF[i]:NOFF[i] + nn],
                        in_=stage[buf][:, s, 0:nn],
                    ).then_inc(osem[buf][s], 16)
            for j in range(2):
                cnt = len([i for i in range(NT) if i % 2 == j])
                for s in range(SLABS):
                    sync.wait_ge(osem[j][s], cnt * 16)

        @block.gpsimd
        def _(gpsimd):
            gpsimd.load_library(mlp)
            for i in range(NT):
                last = i == NT - 1
                npos = TN[i] * K
                if i in CHUNK_WAIT:
                    gpsimd.wait_ge(isem[CHUNK_WAIT[i]], 16)
                if i >= GBUFS and not last:
                    gpsimd.wait_ge(mm_sem, SLABS * (i - GBUFS + 1))
                sl = slice(NOFF[i], NOFF[i + 1])
                gpsimd.dma_gather(
                    out_ap=gbuf(i)[:, :, :],
                    in_ap=table[S * i:S * i + WIN, :],
                    idxs_ap=idx_sb[:, sl],
                    num_idxs=npos,
                    num_idxs_reg=npos,
                    elem_size=ELEM,
                    elem_step=ELEM,
                    transpose=True,
                    single_packet=False,
                ).then_inc(glsem if last else gsem[i % GBUFS], 16)

        @block.vector
        def _(vector):
            vector.wait_ge(bsem, 16)
            for i in range(NT):
                buf = i % 2
                nn = TN[i]
                for s in range(SLABS):
                    vector.wait_ge(mm_sem, SLABS * i + s + 1)
                    if i >= 2:
                        vector.wait_ge(osem[buf][s], 16 * ((i - 2) // 2 + 1))
                    vector.tensor_add(
                        stage[buf][:, s, 0:nn],
                        pbuf(i)[s][:, 0:nn],
                        bias_sb[:, :].to_broadcast([128, nn]),
                    ).then_inc(evac_sem, 1)

        @block.tensor
        def _(tensor):
            tensor.wait_ge(wsem, 16)
            for i in range(NT):
                last = i == NT - 1
                nn = TN[i]
                if last:
                    tensor.wait_ge(glsem, 16)
                else:
                    tensor.wait_ge(gsem[i % GBUFS], 16 * (i // GBUFS + 1))
                if i >= 2:
                    tensor.wait_ge(evac_sem, SLABS * (i - 1))
                src = gbuf(i)
                ps = pbuf(i)
                for s in range(SLABS):
                    for k in range(K):
                        ins = tensor.matmul(
                            ps[s][:, 0:nn],
                            w_sb[:, k * 128:(k + 1) * 128],
                            src[:, s, k * nn:(k + 1) * nn],
                            start=(k == 0),
                            stop=(k == K - 1),
                        )
                    ins.then_inc(mm_sem, 1)

    nc.compile()
    return nc


# ---------------------------------------------------------------- host side
_TILE_OF_NODE = np.zeros(NPAD, dtype=np.int32)
for _t in range(NT):
    _TILE_OF_NODE[NOFF[_t]:NOFF[_t + 1]] = _t


def _assign_slots(refs):
    """refs: [NPAD, K] int32 vnodes. Returns (slot[N+1], spill dict v->{t:slot}).

    Each referenced vnode needs one table slot usable by every tile that
    references it: slot in [S*t_max, S*t_min + WIN). Greedy by deadline with
    union-find next-free; rare infeasible nodes fall back to per-tile slots.
    """
    tile_of_pos = np.repeat(_TILE_OF_NODE, K)
    vn = refs.reshape(-1)

    tmin = np.full(N + 1, NT, dtype=np.int32)
    tmax = np.full(N + 1, -1, dtype=np.int32)
    np.minimum.at(tmin, vn, tile_of_pos)
    np.maximum.at(tmax, vn, tile_of_pos)
    nodes = np.nonzero(tmax >= 0)[0]
    lo = (S * tmax[nodes]).astype(np.int64)
    hi = (S * tmin[nodes] + WIN).astype(np.int64)
    order = np.argsort(hi, kind="stable")

    parent = np.arange(R + 1, dtype=np.int64)

    def find(x):
        root = x
        while parent[root] != root:
            root = parent[root]
        while parent[x] != root:
            parent[x], x = root, parent[x]
        return root

    slot = np.full(N + 1, -1, dtype=np.int64)
    spilled = []
    for i in order:
        f = find(lo[i])
        if f >= hi[i]:
            spilled.append(nodes[i])
            continue
        slot[nodes[i]] = f
        parent[f] = f + 1

    spill = {}
    for v in spilled:
        # per-tile fallback: give the node a slot inside each referencing
        # tile's own window (always feasible while the table isn't full)
        spill[v] = {}
        for t in np.unique(tile_of_pos[vn == v]):
            f = find(S * t)
            if f >= S * t + WIN:
                raise RuntimeError("gather table window full")
            spill[v][int(t)] = int(f)
            parent[f] = f + 1
    return slot, spill


def _pack_inputs(batch, neighborhoods, kernel, bias):
    batch = np.asarray(batch, dtype=np.float32)
    nb = np.asarray(neighborhoods, dtype=np.int64).astype(np.int32)
    w = np.asarray(kernel, dtype=np.float32)
    bias = np.asarray(bias, dtype=np.float32).reshape(COUT)

    # node content rows: (N, 256) bf16, unit u = batch b=u//64, channel u%64
    noderows = batch.transpose(1, 0, 2).reshape(N, ELEM).astype(BF)

    # block-diag stationary weight tiles [128, K*128]
    wt = np.zeros((128, K, 128), dtype=BF)
    wbf = w.astype(BF)
    for k in range(K):
        wt[0:64, k, 0:64] = wbf[k]
        wt[64:128, k, 64:128] = wbf[k]
    wt = wt.reshape(128, K * 128)

    bias_t = np.tile(bias, 2).reshape(128, 1).astype(np.float32)

    tables = []
    idx_maps = []
    for c in range(NCORES):
        n0 = c * NODES_PER_CORE
        refs = np.full((NPAD, K), ZERO, dtype=np.int32)
        refs[:NODES_PER_CORE] = nb[n0:n0 + NODES_PER_CORE]

        slot, spill = _assign_slots(refs)

        table = np.zeros((R, ELEM), dtype=BF)
        assigned = np.nonzero(slot >= 0)[0]
        real = assigned[assigned < N]
        table[slot[real]] = noderows[real]
        for v, tslots in spill.items():
            if v < N:
                for t, f in tslots.items():
                    table[f] = noderows[v]

        idx = slot[refs] - (S * _TILE_OF_NODE[:, None]).astype(np.int64)
        for v, tslots in spill.items():
            mask = refs == v
            for t, f in tslots.items():
                rows = slice(NOFF[t], NOFF[t + 1])
                idx[rows][mask[rows]] = f - S * t
        assert idx.min() >= 0 and idx.max() < WIN, (idx.min(), idx.max())
        idx = idx.astype(np.int16)

        # wrap positions k-major per tile -> [128, NPAD] int16
        cols = []
        for t in range(NT):
            nn = TN[t]
            a = idx[NOFF[t]:NOFF[t + 1]]                 # [nn, K]
            a = a.T.reshape(nn * K)                      # k-major positions
            a = a.reshape(nn * K // 16, 16).T            # [16, cols]
            cols.append(np.tile(a, (8, 1)))              # [128, cols]
        idx_maps.append(np.ascontiguousarray(np.concatenate(cols, axis=1)))
        tables.append(table)

    return tables, wt, bias_t, idx_maps


_PROGRAM_CACHE = {}


def _run(batch, neighborhoods, kernel, bias, **spmd_kwargs):
    tables, wt, bias_t, idx_maps = _pack_inputs(batch, neighborhoods, kernel, bias)

    if "nc" not in _PROGRAM_CACHE:
        _PROGRAM_CACHE["nc"] = build_program()
    nc = _PROGRAM_CACHE["nc"]

    in_maps = []
    for c in range(NCORES):
        in_maps.append({
            "table": tables[c],
            "idx": idx_maps[c],
            "w": wt,
            "biasx": bias_t,
        })

    kres = run_bass_kernel_spmd(nc, in_maps, list(range(NCORES)), **spmd_kwargs)
    res = kres.results

    out = np.empty((B, N, COUT), dtype=np.float32)
    for c in range(NCORES):
        o = np.asarray(res[c]["out"])[:, :, :NODES_PER_CORE]   # [S, 128, n]
        o = o.astype(np.float32)
        o = o.reshape(SLABS, 2, COUT, NODES_PER_CORE)          # [s, b01, o, n]
        o = o.transpose(0, 1, 3, 2).reshape(B, NODES_PER_CORE, COUT)
        out[:, c * NODES_PER_CORE:(c + 1) * NODES_PER_CORE, :] = o
    return out, kres


def kernel(batch, neighborhoods, kernel, bias):
    out, _ = _run(batch, neighborhoods, kernel, bias)
    return out
